# revision 1
# baseline (speedup 1.0000x reference)
"""Trainium2 Bass kernel for nn_CombinedModel (NMS detection + ROI classifier).

Sharding: pooled-pixel-row sharding. Core c computes conv output rows
y in [8c, 8c+8) (= pooled rows py in [4c,4c+4)) of ALL 300 ROIs, which is
exactly the k-slice S_c = {(oc, py, px): py in [4c,4c+4)} of the 16384-wide
W1 contraction. Each core runs the 8-head GEMM against its 2048-row W1
slice, a ReduceScatter sums the partial [8,128,304] and hands head c to
core c, which applies bias/relu + its head matmul + keep mask.
NMS / top-k / ROI selection is tiny and done host-side during input prep.
"""
import numpy as np

N_CORES = 8
R = 304            # 300 rois padded to 8*38
IMG = 640
INP = 64
CONF = 0.25
IOU = 0.45
K = 300
PROV, ALPHA, AD = 38, 25, 35
OUTW = 40          # padded per-core head width

_CACHE = {}


def _build_bass():
    import concourse.bacc as bacc
    import concourse.mybir as mybir
    import concourse.tile as tile

    nc = bacc.Bacc("TRN2", target_bir_lowering=False, debug=False,
                   num_devices=N_CORES)
    f32 = mybir.dt.float32
    cols = nc.dram_tensor("cols", [108, 38912], f32, kind="ExternalInput").ap()
    wstk = nc.dram_tensor("wstk", [108, 64], f32, kind="ExternalInput").ap()
    bc64 = nc.dram_tensor("bc64", [64, 1], f32, kind="ExternalInput").ap()
    w1s = nc.dram_tensor("w1s", [8, 16, 128, 128], f32, kind="ExternalInput").ap()
    b1c = nc.dram_tensor("b1c", [128, 1], f32, kind="ExternalInput").ap()
    w2 = nc.dram_tensor("w2", [128, OUTW], f32, kind="ExternalInput").ap()
    b2 = nc.dram_tensor("b2", [OUTW, 1], f32, kind="ExternalInput").ap()
    keepm = nc.dram_tensor("keepm", [OUTW, R], f32, kind="ExternalInput").ap()
    out = nc.dram_tensor("out", [OUTW, R], f32, kind="ExternalOutput").ap()

    NM = 76  # conv matmuls of 512 cols each

    with tile.TileContext(nc) as tc:
        with (
            tc.tile_pool(name="const", bufs=1) as cpool,
            tc.tile_pool(name="colsp", bufs=3) as colsp,
            tc.tile_pool(name="psum", bufs=1, space="PSUM") as psum,
            tc.tile_pool(name="work", bufs=2) as work,
            tc.tile_pool(name="dram", bufs=1, space="DRAM") as dpool,
        ):
            wstk_sb = cpool.tile([108, 64], f32)
            nc.sync.dma_start(wstk_sb[:], wstk[:])
            bc64_sb = cpool.tile([64, 1], f32)
            nc.sync.dma_start(bc64_sb[:], bc64[:])
            b1c_sb = cpool.tile([128, 1], f32)
            nc.sync.dma_start(b1c_sb[:], b1c[:])
            w2_sb = cpool.tile([128, OUTW], f32)
            nc.sync.dma_start(w2_sb[:], w2[:])
            b2_sb = cpool.tile([OUTW, 1], f32)
            nc.sync.dma_start(b2_sb[:], b2[:])
            keep_sb = cpool.tile([OUTW, R], f32)
            nc.sync.dma_start(keep_sb[:], keepm[:])

            pooled2 = cpool.tile([128, 16, R], f32)

            # conv + pool: 4 col chunks of 19 matmuls each
            CH = 19
            for ch in range(4):
                ctile = colsp.tile([108, CH * 512], f32, tag="cols", bufs=2)
                nc.sync.dma_start(ctile[:], cols[:, ch * CH * 512:(ch + 1) * CH * 512])
                for j in range(CH):
                    m = ch * CH + j
                    ps = psum.tile([64, 4, 2, 64], f32, tag="cv", bufs=4)
                    nc.tensor.matmul(ps.rearrange("p a b c -> p (a b c)"),
                                     wstk_sb[:], ctile[:, j * 512:(j + 1) * 512],
                                     start=True, stop=True)
                    craw = work.tile([64, 4, 2, 64], f32, tag="craw")
                    nc.scalar.activation(
                        craw.rearrange("p a b c -> p (a b c)"),
                        ps.rearrange("p a b c -> p (a b c)"),
                        mybir.ActivationFunctionType.Relu,
                        bias=bc64_sb[:])
                    t0 = work.tile([64, 4, 32], f32, tag="t0")
                    t1 = work.tile([64, 4, 32], f32, tag="t1")
                    nc.vector.tensor_tensor(out=t0[:], in0=craw[:, :, 0, 0::2],
                                            in1=craw[:, :, 0, 1::2],
                                            op=mybir.AluOpType.max)
                    nc.vector.tensor_tensor(out=t1[:], in0=craw[:, :, 1, 0::2],
                                            in1=craw[:, :, 1, 1::2],
                                            op=mybir.AluOpType.max)
                    nc.vector.tensor_tensor(
                        out=pooled2[0:64, :, 4 * m:4 * m + 4].rearrange(
                            "p x r -> p r x"),
                        in0=t0[:, :, 0::2], in1=t1[:, :, 0::2],
                        op=mybir.AluOpType.max)
                    nc.vector.tensor_tensor(
                        out=pooled2[64:128, :, 4 * m:4 * m + 4].rearrange(
                            "p x r -> p r x"),
                        in0=t0[:, :, 1::2], in1=t1[:, :, 1::2],
                        op=mybir.AluOpType.max)

            # 8-head GEMM over this core's 2048-row W1 slice
            import os
            STAGE = int(os.environ.get("KSTAGE", "3"))

            if STAGE == 0:
                om0 = work.tile([OUTW, R], f32, tag="om")
                nc.vector.tensor_copy(om0[:], pooled2[:OUTW, 0, :])
                nc.sync.dma_start(out[:], om0[:])
            if STAGE >= 1:
                parts = cpool.tile([128, 8, R], f32)
                for h in range(8):
                    w1h = colsp.tile([128, 16, 128], f32, tag="w1h", bufs=2)
                    nc.sync.dma_start(w1h[:], w1s[h].rearrange("q k d -> k q d"))
                    ph = psum.tile([128, R], f32, tag="gemm", bufs=2)
                    for q in range(16):
                        nc.tensor.matmul(ph[:], w1h[:, q, :], pooled2[:, q, :],
                                         start=(q == 0), stop=(q == 15))
                    nc.vector.tensor_copy(parts[:, h, :], ph[:])
            if STAGE == 1:
                om1 = work.tile([OUTW, R], f32, tag="om")
                nc.vector.tensor_copy(om1[:], parts[:OUTW, 0, :])
                nc.sync.dma_start(out[:], om1[:])
            if STAGE >= 2:
                cc_in = dpool.tile([8, 128, R], f32)
                cc_out = dpool.tile([128, R], f32)
                nc.sync.dma_start(cc_in.rearrange("h p r -> p h r"), parts[:])
                nc.gpsimd.collective_compute(
                    "ReduceScatter", mybir.AluOpType.add,
                    ins=[cc_in[:]], outs=[cc_out[:]],
                    replica_groups=[list(range(N_CORES))],
                )
                hsb = work.tile([128, R], f32, tag="hsb")
                nc.sync.dma_start(hsb[:], cc_out[:])
                hrelu = work.tile([128, R], f32, tag="hrelu")
                nc.scalar.activation(hrelu[:], hsb[:],
                                     mybir.ActivationFunctionType.Relu,
                                     bias=b1c_sb[:])
                po = psum.tile([OUTW, R], f32, tag="head")
                nc.tensor.matmul(po[:], w2_sb[:], hrelu[:], start=True, stop=True)
                ob = work.tile([OUTW, R], f32, tag="ob")
                nc.vector.tensor_scalar(ob[:], po[:], b2_sb[:], None,
                                        op0=mybir.AluOpType.add)
                om = work.tile([OUTW, R], f32, tag="om")
                nc.vector.tensor_tensor(out=om[:], in0=ob[:], in1=keep_sb[:],
                                        op=mybir.AluOpType.mult)
                nc.sync.dma_start(out[:], om[:])
    nc.compile()
    return nc


def _host_prep(preds, image, W_conv, b_conv, W1, b1, W2p, b2p, W2a, b2a, W2d, b2d):
    p = preds[0].astype(np.float32)
    score = p[:, 4] * p[:, 5]
    masked = np.where(score > CONF, score, -np.inf)
    idx = np.argsort(-masked, kind="stable")[:K]
    top_s = masked[idx]
    xy, wh = p[:, 0:2], p[:, 2:4]
    boxes = np.concatenate([xy - wh / 2, xy + wh / 2], axis=-1)
    b = boxes[idx]
    valid = top_s > CONF
    x1, y1, x2, y2 = b[:, 0], b[:, 1], b[:, 2], b[:, 3]
    area = (x2 - x1) * (y2 - y1)
    iw = np.clip(np.minimum(x2[:, None], x2[None, :]) - np.maximum(x1[:, None], x1[None, :]), 0, None)
    ih = np.clip(np.minimum(y2[:, None], y2[None, :]) - np.maximum(y1[:, None], y1[None, :]), 0, None)
    iou = iw * ih / (area[:, None] + area[None, :] - iw * ih + 1e-7)
    keep = valid.copy()
    for i in range(K):
        sup = np.any((iou[i, :i] > IOU) & keep[:i])
        keep[i] = keep[i] & ~sup

    xi = np.clip(np.round(b[:, 0]).astype(np.int32), 0, IMG - INP)
    yi = np.clip(np.round(b[:, 1]).astype(np.int32), 0, IMG - INP)
    img0 = image[0]
    pad = np.zeros((R, 3, 66, 66), np.float32)
    for r in range(K):
        pad[r, :, 1:65, 1:65] = img0[:, yi[r]:yi[r] + 64, xi[r]:xi[r] + 64]

    from numpy.lib.stride_tricks import sliding_window_view
    # patches[roi, c, yy, x, ky, kx]
    patches = sliding_window_view(pad, (3, 3), axis=(2, 3))
    P2 = np.ascontiguousarray(patches.transpose(2, 1, 4, 5, 0, 3))  # [yy,c,ky,kx,roi,x]
    cols_all = np.ascontiguousarray(
        P2.reshape(8, 4, 2, 27, R, 64).transpose(0, 1, 3, 4, 2, 5)
    ).reshape(8, 108, 38912)

    wstk = np.zeros((108, 64), np.float32)
    wc = W_conv.reshape(16, 27).T  # [27, 16]
    for ph in range(4):
        wstk[ph * 27:(ph + 1) * 27, ph * 16:(ph + 1) * 16] = wc
    bc64 = np.tile(b_conv.astype(np.float32), 4).reshape(64, 1)

    # w1s[core][h, px, py*16+oc, d]
    W1r = W1.reshape(8, 16, 32, 32, 128)  # [h, oc, py, px, d]
    w1s_all = np.empty((8, 8, 16, 128, 128), np.float32)
    for core in range(8):
        blk = W1r[:, :, 4 * core:4 * core + 4, :, :]       # [h, oc, py4, px, d]
        t = np.ascontiguousarray(blk.transpose(0, 3, 2, 1, 4))  # [h, px, py, oc, d]
        w1s_all[core] = t.reshape(8, 16, 2, 64, 128).reshape(8, 16, 128, 128)

    w2_all = np.zeros((8, 128, OUTW), np.float32)
    b2_all = np.zeros((8, OUTW, 1), np.float32)
    w2_all[0, :, :PROV] = W2p; b2_all[0, :PROV, 0] = b2p
    w2_all[1, :, :ALPHA] = W2a; b2_all[1, :ALPHA, 0] = b2a
    for j in range(6):
        w2_all[2 + j, :, :AD] = W2d[j]; b2_all[2 + j, :AD, 0] = b2d[j]

    keepf = np.zeros((R,), np.float32)
    keepf[:K] = keep.astype(np.float32)
    keepm = np.broadcast_to(keepf, (OUTW, R)).copy()

    in_maps = []
    for core in range(8):
        in_maps.append({
            "cols": cols_all[core],
            "wstk": wstk,
            "bc64": bc64,
            "w1s": w1s_all[core],
            "b1c": b1[core].reshape(128, 1).astype(np.float32),
            "w2": w2_all[core],
            "b2": b2_all[core],
            "keepm": keepm,
        })
    return in_maps


def kernel(**inputs):
    from concourse import bass_utils
    if "nc" not in _CACHE:
        _CACHE["nc"] = _build_bass()
    nc = _CACHE["nc"]
    in_maps = _host_prep(**{k: np.asarray(v) for k, v in inputs.items()})
    res = bass_utils.run_bass_kernel_spmd(nc, in_maps, core_ids=list(range(N_CORES)))
    _CACHE["last_res"] = res
    outs = [res.results[c]["out"].T for c in range(N_CORES)]  # [304, 40] each
    logits = np.concatenate(
        [outs[0][:K, :PROV], outs[1][:K, :ALPHA]]
        + [outs[2 + j][:K, :AD] for j in range(6)], axis=1)
    return logits.astype(np.float32)



# revision 20
# speedup vs baseline: 2.0746x; 2.0746x over previous
"""Trainium2 Bass kernel for nn_CombinedModel (NMS detection + ROI classifier).

Sharding: pooled-pixel-row sharding. Core c computes conv output rows
y in [8c, 8c+8) (= pooled rows py in [4c,4c+4)) of ALL 300 ROIs, which is
exactly the k-slice of the 16384-wide W1 contraction. Each core runs the
8-head GEMM against its 2048-row W1 slice; two R-chunked bf16
ReduceScatters sum the partial [8,128,R] and hand head c to core c, which
applies bias/relu + its head matmul + keep mask.
NMS / top-k / ROI selection is tiny and done host-side during input prep.

Perf notes vs v1 (261us):
- all matmuls bf16 (1 cyc/row vs 4 for fp32), weights/cols/W1 cast host-side
- b_conv folded into the conv matmul via a 28th ones-row per im2col group
- conv matmuls use all 128 out-partitions: per 512-col unit, two matmuls
  with left/right-zero-padded stationary tensors write the pooled-px
  parity s to the low/high 64 partitions, so PSUM partitions equal the
  GEMM k-layout and one DVE XY max-reduce over (y, x-pair) does the
  whole 2x2 pool; bias+relu deferred to one Act op per R-chunk
- drains alternate DVE-direct reduce / Act-copy-then-reduce (DVE and
  Pool ops may touch at most one PSUM operand; Pool none at all)
- W1 slice kept SBUF-resident (32KB/partition), streamed per-head
- GEMM split in two R-chunks; ReduceScatter of chunk 0 overlaps chunk 1
- cols streamed in 19 chunks alternating the SP/Act DMA queues
"""
import numpy as np

N_CORES = 8
R = 304            # 300 rois padded to 19*16
IMG = 640
INP = 64
CONF = 0.25
IOU = 0.45
K = 300
PROV, ALPHA, AD = 38, 25, 35
OUTW = 40          # padded per-core head width
RCHUNKS = [(0, 160), (160, 304)]
QUADS = [4, 4, 2, 4, 4, 1]   # col-chunks per op2 quad group (sum 19)

_CACHE = {}


def _build_bass():
    import concourse.bacc as bacc
    import concourse.mybir as mybir
    import concourse.tile as tile

    nc = bacc.Bacc("TRN2", target_bir_lowering=False, debug=False,
                   num_devices=N_CORES)
    f32 = mybir.dt.float32
    bf16 = mybir.dt.bfloat16
    MAX = mybir.AluOpType.max

    cols = nc.dram_tensor("cols", [112, 38912], bf16, kind="ExternalInput").ap()
    wsab = nc.dram_tensor("wsab", [112, 256], bf16, kind="ExternalInput").ap()
    w1s = nc.dram_tensor("w1s", [128, 8, 16, 128], bf16, kind="ExternalInput").ap()
    b1c = nc.dram_tensor("b1c", [128, 1], f32, kind="ExternalInput").ap()
    bcc = nc.dram_tensor("bcc", [128, 1], f32, kind="ExternalInput").ap()
    w2 = nc.dram_tensor("w2", [128, OUTW], bf16, kind="ExternalInput").ap()
    b2 = nc.dram_tensor("b2", [OUTW, 1], f32, kind="ExternalInput").ap()
    keepm = nc.dram_tensor("keepm", [OUTW, R], bf16, kind="ExternalInput").ap()
    out = nc.dram_tensor("out", [OUTW, R], f32, kind="ExternalOutput").ap()

    with tile.TileContext(nc) as tc:
        with (
            tc.tile_pool(name="const", bufs=1) as cpool,
            tc.tile_pool(name="colsp", bufs=2) as colsp,
            tc.tile_pool(name="psum", bufs=1, space="PSUM") as psum,
            tc.tile_pool(name="work", bufs=2) as work,
            tc.tile_pool(name="dram", bufs=1, space="DRAM") as dpool,
        ):
            wsab_sb = cpool.tile([112, 2, 128], bf16)
            nc.scalar.dma_start(wsab_sb.rearrange("p a b -> p (a b)"), wsab[:])
            b1c_sb = cpool.tile([128, 1], f32)
            nc.scalar.dma_start(b1c_sb[:], b1c[:])
            bcc_sb = cpool.tile([128, 1], f32)
            nc.scalar.dma_start(bcc_sb[:], bcc[:])
            w2_sb = cpool.tile([128, OUTW], bf16)
            nc.scalar.dma_start(w2_sb[:], w2[:])
            b2_sb = cpool.tile([OUTW, 1], f32)
            nc.scalar.dma_start(b2_sb[:], b2[:])
            keep_sb = cpool.tile([OUTW, R], bf16)
            nc.scalar.dma_start(keep_sb[:], keepm[:])

            pooled2 = cpool.tile([128, 16, R], bf16)
            # view exposing col = C*16 + u*4 + cg*2 + rl
            pooled_v = pooled2.rearrange("p q (C u g r) -> p q C u g r",
                                         C=19, u=4, g=2, r=2)
            w1all = cpool.tile([128, 8, 16, 128], bf16)

            # skew barrier: a tiny collective that completes while conv
            # runs, so the real RS doesn't absorb cross-core start skew
            dmy_in = dpool.tile([8, 16], f32, tag="dmy_in")
            dmy_out = dpool.tile([16], f32, tag="dmy_out")
            nc.gpsimd.collective_compute(
                "ReduceScatter", mybir.AluOpType.add,
                ins=[dmy_in[:]], outs=[dmy_out[:]],
                replica_groups=[list(range(N_CORES))],
            )

            # ---- conv + pool: 19 chunks of 4 units (512 cols each) ----
            # Column order per unit: (s | q, roi, y, e); the A/B stationary
            # halves put s (pooled-px parity) in the partition dim, so PSUM
            # partitions == pooled2 partitions and one XY max-reduce over
            # (y, e) performs the whole 2x2 pool.
            pooled_u = pooled2.rearrange("p q (C u r) -> p q C u r",
                                         C=19, u=4, r=4)
            for ch in range(19):
                ctile = colsp.tile([112, 4, 2, 256], bf16, tag="cols", bufs=3)
                eng_dma = nc.sync if ch % 2 == 0 else nc.scalar
                eng_dma.dma_start(ctile.rearrange("p a b c -> p (a b c)"),
                                  cols[:, ch * 2048:(ch + 1) * 2048])
                P = psum.tile([128, 4, 256], f32, tag="cv", bufs=3)
                for u in range(4):
                    nc.tensor.matmul(P[:, u, :], wsab_sb[:, 0, :],
                                     ctile[:, u, 0, :], start=True, stop=False)
                    nc.tensor.matmul(P[:, u, :], wsab_sb[:, 1, :],
                                     ctile[:, u, 1, :], start=False, stop=True)
                pout = pooled_u[:, :, ch, :, :].rearrange("p q u r -> p u q r")
                if ch % 3 == 0:
                    nc.vector.tensor_reduce(
                        pout,
                        P.rearrange("p u (q r y e) -> p (u q r) y e",
                                    q=16, r=4, y=2, e=2),
                        mybir.AxisListType.XY, MAX)
                else:
                    craw = work.tile([128, 4, 16, 4, 2, 2], bf16, tag="craw",
                                     bufs=3)
                    nc.scalar.activation(
                        craw.rearrange("p u q r y e -> p (u q r y e)"),
                        P.rearrange("p u x -> p (u x)"),
                        mybir.ActivationFunctionType.Copy)
                    nc.vector.tensor_reduce(
                        pout,
                        craw.rearrange("p u q r y e -> p (u q r) y e"),
                        mybir.AxisListType.XY, MAX)

            # W1 slice prefetch (after cols DMAs on the Act queue)
            for h in range(8):
                nc.scalar.dma_start(w1all[:, h], w1s[:, h])

            # ---- 8-head GEMM in two R-chunks + chunked ReduceScatter ----
            relu = mybir.ActivationFunctionType.Relu
            cc_outs = []
            for ci, (c0, c1) in enumerate(RCHUNKS):
                cw = c1 - c0
                # deferred conv bias + relu for this chunk's columns
                nc.scalar.activation(pooled2[:, :, c0:c1],
                                     pooled2[:, :, c0:c1],
                                     relu, bias=bcc_sb[:])
                parts = work.tile([128, 8, cw], bf16, tag=f"parts{ci}", bufs=1)
                for h in range(8):
                    ph = psum.tile([128, R], f32, tag="gemm", bufs=2)
                    for q in range(16):
                        nc.tensor.matmul(ph[:, 0:cw], w1all[:, h, q, :],
                                         pooled2[:, q, c0:c1],
                                         start=(q == 0), stop=(q == 15))
                    # gpsimd can't read PSUM; split copies between DVE and Act
                    if h % 2 == 0:
                        nc.vector.tensor_copy(parts[:, h, :], ph[:, 0:cw])
                    else:
                        nc.scalar.activation(parts[:, h, :], ph[:, 0:cw],
                                             mybir.ActivationFunctionType.Copy)
                cc_in = dpool.tile([8, 128, cw], bf16, tag=f"cc_in{ci}")
                cc_out = dpool.tile([128, cw], bf16, tag=f"cc_out{ci}")
                nc.sync.dma_start(cc_in.rearrange("h p r -> p h r"), parts[:])
                nc.gpsimd.collective_compute(
                    "ReduceScatter", mybir.AluOpType.add,
                    ins=[cc_in[:]], outs=[cc_out[:]],
                    replica_groups=[list(range(N_CORES))],
                )
                cc_outs.append(cc_out)

            # ---- per-chunk tail: h = relu(sum + b1); head matmul; mask ----
            for ci, (c0, c1) in enumerate(RCHUNKS):
                cw = c1 - c0
                hsb = work.tile([128, R], bf16, tag="hsb", bufs=2)
                nc.scalar.dma_start(hsb[:, 0:cw], cc_outs[ci][:])
                hrelu = work.tile([128, R], bf16, tag="hrelu", bufs=2)
                nc.scalar.activation(hrelu[:, 0:cw], hsb[:, 0:cw],
                                     relu, bias=b1c_sb[:])
                po = psum.tile([128, R], f32, tag="gemm", bufs=2)
                nc.tensor.matmul(po[0:OUTW, 0:cw], w2_sb[:], hrelu[:, 0:cw],
                                 start=True, stop=True)
                om = work.tile([OUTW, R], f32, tag="om", bufs=2)
                nc.vector.scalar_tensor_tensor(
                    out=om[:, 0:cw], in0=po[0:OUTW, 0:cw], scalar=b2_sb[:],
                    in1=keep_sb[:, c0:c1],
                    op0=mybir.AluOpType.add, op1=mybir.AluOpType.mult)
                nc.sync.dma_start(out[:, c0:c1], om[:, 0:cw])
    nc.compile()
    return nc


def _host_prep(preds, image, W_conv, b_conv, W1, b1, W2p, b2p, W2a, b2a, W2d, b2d):
    import ml_dtypes
    bf = ml_dtypes.bfloat16
    p = preds[0].astype(np.float32)
    score = p[:, 4] * p[:, 5]
    masked = np.where(score > CONF, score, -np.inf)
    idx = np.argsort(-masked, kind="stable")[:K]
    top_s = masked[idx]
    xy, wh = p[:, 0:2], p[:, 2:4]
    boxes = np.concatenate([xy - wh / 2, xy + wh / 2], axis=-1)
    b = boxes[idx]
    valid = top_s > CONF
    x1, y1, x2, y2 = b[:, 0], b[:, 1], b[:, 2], b[:, 3]
    area = (x2 - x1) * (y2 - y1)
    iw = np.clip(np.minimum(x2[:, None], x2[None, :]) - np.maximum(x1[:, None], x1[None, :]), 0, None)
    ih = np.clip(np.minimum(y2[:, None], y2[None, :]) - np.maximum(y1[:, None], y1[None, :]), 0, None)
    iou = iw * ih / (area[:, None] + area[None, :] - iw * ih + 1e-7)
    keep = valid.copy()
    for i in range(K):
        sup = np.any((iou[i, :i] > IOU) & keep[:i])
        keep[i] = keep[i] & ~sup

    xi = np.clip(np.round(b[:, 0]).astype(np.int32), 0, IMG - INP)
    yi = np.clip(np.round(b[:, 1]).astype(np.int32), 0, IMG - INP)
    img0 = image[0]
    pad = np.zeros((R, 3, 66, 66), np.float32)
    for r in range(K):
        pad[r, :, 1:65, 1:65] = img0[:, yi[r]:yi[r] + 64, xi[r]:xi[r] + 64]

    from numpy.lib.stride_tricks import sliding_window_view
    # patches[roi, c, yy, x, ky, kx]
    patches = sliding_window_view(pad, (3, 3), axis=(2, 3))
    P2 = np.ascontiguousarray(patches.transpose(2, 1, 4, 5, 0, 3))  # [yy,c,ky,kx,roi,x]
    # [core, ygrp4, 27, R, ysub2, x64] -> 28-row groups with a ones row,
    # then permute x64 -> (s2, q16, e2) with x = 4q + 2s + e so the pool
    # window is the innermost psum column dim
    c27 = P2.reshape(8, 4, 2, 27, R, 64).transpose(0, 1, 3, 4, 2, 5)
    cols28 = np.ones((8, 4, 28, R, 2, 64), np.float32)
    cols28[:, :, 0:27] = c27
    # unit column order: (s2 | q16, roi4, y2, e2), x = 4q + 2s + e
    colsx = cols28.reshape(8, 4, 28, 76, 4, 2, 16, 2, 2).transpose(
        0, 1, 2, 3, 7, 6, 4, 5, 8)       # [.., 76u, s2, q16, roi4, y2, e2]
    cols_all = np.ascontiguousarray(colsx).reshape(8, 112, 38912).astype(bf)

    # stationary pair: [112, 2, 128]; group rows 0:27 = wc, row 27 = b_conv
    wstk = np.zeros((112, 64), np.float32)
    wc = W_conv.reshape(16, 27).T  # [27, 16]
    for g in range(4):
        wstk[g * 28:g * 28 + 27, g * 16:(g + 1) * 16] = wc
        wstk[g * 28 + 27, g * 16:(g + 1) * 16] = b_conv
    wsab = np.zeros((112, 2, 128), np.float32)
    wsab[:, 0, 0:64] = wstk
    wsab[:, 1, 64:128] = wstk
    wsab = wsab.reshape(112, 256).astype(bf)

    # w1s[core]: [k=128, h, q, d] with k = s*64 + py*16 + oc, q = px//2
    W1r = W1.reshape(8, 16, 32, 32, 128)  # [h, oc, py, px, d]
    w1s_all = np.empty((8, 128, 8, 16, 128), np.float32)
    for core in range(8):
        blk = W1r[:, :, 4 * core:4 * core + 4, :, :]       # [h, oc, py4, px32, d]
        t = np.ascontiguousarray(blk.transpose(0, 3, 2, 1, 4))  # [h, px, py, oc, d]
        t = t.reshape(8, 16, 2, 64, 128)                   # [h, q, s, py*oc, d]
        t = t.transpose(0, 1, 2, 3, 4).reshape(8, 16, 128, 128)  # [h, q, k, d]
        w1s_all[core] = t.transpose(2, 0, 1, 3)            # [k, h, q, d]
    w1s_all = w1s_all.astype(bf)

    bcc = np.tile(b_conv.astype(np.float32), 8).reshape(128, 1)

    w2_all = np.zeros((8, 128, OUTW), np.float32)
    b2_all = np.zeros((8, OUTW, 1), np.float32)
    w2_all[0, :, :PROV] = W2p; b2_all[0, :PROV, 0] = b2p
    w2_all[1, :, :ALPHA] = W2a; b2_all[1, :ALPHA, 0] = b2a
    for j in range(6):
        w2_all[2 + j, :, :AD] = W2d[j]; b2_all[2 + j, :AD, 0] = b2d[j]

    keepf = np.zeros((R,), np.float32)
    keepf[:K] = keep.astype(np.float32)
    keepm = np.broadcast_to(keepf, (OUTW, R)).astype(bf).copy()

    in_maps = []
    for core in range(8):
        in_maps.append({
            "cols": cols_all[core],
            "wsab": wsab,
            "w1s": w1s_all[core],
            "b1c": b1[core].reshape(128, 1).astype(np.float32),
            "bcc": bcc,
            "w2": w2_all[core].astype(bf),
            "b2": b2_all[core],
            "keepm": keepm,
        })
    return in_maps


def kernel(**inputs):
    from concourse import bass_utils
    if "nc" not in _CACHE:
        _CACHE["nc"] = _build_bass()
    nc = _CACHE["nc"]
    in_maps = _host_prep(**{k: np.asarray(v) for k, v in inputs.items()})
    res = bass_utils.run_bass_kernel_spmd(nc, in_maps, core_ids=list(range(N_CORES)))
    _CACHE["last_res"] = res
    outs = [np.asarray(res.results[c]["out"]).T for c in range(N_CORES)]  # [304, 40]
    logits = np.concatenate(
        [outs[0][:K, :PROV], outs[1][:K, :ALPHA]]
        + [outs[2 + j][:K, :AD] for j in range(6)], axis=1)
    return logits.astype(np.float32)
